# revision 17
# baseline (speedup 1.0000x reference)
"""Distributed GQA attention kernel for one TRN2 chip (8 NeuronCores).

Problem: B=2, L=2048, HID=2048, H=32 q-heads, HKV=8 kv-heads, D=64,
rotary embedding, causal softmax, o-proj.

Sharding: core i -> batch b=i//4, TP rank r=i%4.  Each core computes
8 q-heads / 2 kv-heads of its batch, all-gathers the attention outputs
(feature-major, bf16) within its 4-core TP group, then computes its
512 output columns of the o-proj.  Host assembles the full output.

Schedule (v4):
- proj(tt) and attention interleave per q-tile so the scalar-engine exp
  stream overlaps projection matmuls.
- Per key-tile kt, BOTH heads' scores go into ONE [128,1024] PSUM tile
  so the two K=64 quadrant matmuls share one slot-alloc event and issue
  back-to-back under tc.high_priority -> they run CONCURRENTLY on
  disjoint PE row halves.  One exp per kt covers both heads, restricted
  to the causally-valid column range via a strided AP.
- RoPE's partition half-swap runs on the PE (constant permutation
  matrix as stationary operand) instead of DMA, per M-slice, so roped
  Q/K trail the projection chains by ~2.5us instead of ~9us.
- AllGathers: quarters 0/1 ship whole (after proj(1)/proj(2)); quarters
  2/3 ship per-head-pair as each attention group completes, so o-proj
  chunks stream in arrival order and only the last small piece's
  16 matmuls remain after the final AG.
- Queues: sync = weights + ship bounces; scalar = x tiles + Wo;
  gpsimd = rope tables, gather loads, output stores (no head-blocking).
- x arrives as 4 sub-tiles per token tile and Wq as 2 halves so the
  first K-proj matmul can start at ~18us.
- All matmuls bf16 with fp32 PSUM.  Softmax skips the row-max (logits
  bounded ~|6|) and gets row sums free from a 64-wide ones block in V's
  stationary operand; normalization is a DVE reciprocal + multiply.
"""

import sys

sys.path.insert(0, "/opt/trn_rl_repo")

import numpy as np
import ml_dtypes

B, L, HID = 2, 2048, 2048
H, HKV, D = 32, 8, 64
N_CORES = 8
TP = 4           # tensor-parallel group size
HL = 8           # q heads per core
CW = 512         # o-proj output columns per core
TT = 4           # t tiles of 512 over L
CCH = HID // 128 # contraction chunks (16)
XSP = 4          # x sub-tiles per token tile
BF16 = ml_dtypes.bfloat16

_cache = {}


def _build_graph():
    import concourse.bass as bass
    import concourse.tile as tile
    from concourse import bacc, mybir

    dt = mybir.dt
    f32, bf16 = dt.float32, dt.bfloat16

    nc = bacc.Bacc("TRN2", target_bir_lowering=False, debug=False,
                   num_devices=N_CORES)

    # all packed [128, n] with per-partition-contiguous layout
    xP = nc.dram_tensor("xP", [128, TT * CCH * 512], bf16,
                        kind="ExternalInput")
    WqP = nc.dram_tensor("WqP", [128, CCH * 512], bf16, kind="ExternalInput")
    WkP = nc.dram_tensor("WkP", [128, CCH * 128], bf16, kind="ExternalInput")
    WvP = nc.dram_tensor("WvP", [128, CCH * 128], bf16, kind="ExternalInput")
    WoP = nc.dram_tensor("WoP", [128, CCH * 512], bf16, kind="ExternalInput")
    C1 = nc.dram_tensor("C1", [128, L], bf16, kind="ExternalInput")
    C2 = nc.dram_tensor("C2", [128, L], bf16, kind="ExternalInput")
    PM = nc.dram_tensor("PM", [128, 128], bf16, kind="ExternalInput")
    out = nc.dram_tensor("out", [CW, L], f32, kind="ExternalOutput")

    with tile.TileContext(nc) as tc:
        with (
            tc.tile_pool(name="persist", bufs=1) as persist,
            tc.tile_pool(name="mm", bufs=2, space="PSUM") as pmm,
            tc.tile_pool(name="ps_s", bufs=2, space="PSUM") as ps_s,
            tc.tile_pool(name="po", bufs=2, space="PSUM") as po,
            tc.tile_pool(name="pp", bufs=4) as pp,
            tc.tile_pool(name="ost", bufs=2) as ostp,
            tc.tile_pool(name="dram", bufs=1, space="DRAM") as dram,
        ):
            # ---- persistent SBUF tensors ----
            qq = persist.tile([128, HL // 2 * L], bf16)      # roped Q^T, 2MB
            kk = persist.tile([128, L], bf16)                # roped K^T (2 kv)
            v2t = persist.tile([128, L], bf16)               # V^T staging
            v2 = persist.tile([128, CCH * 256], bf16)        # [V|1|V|1] per kt
            ao = persist.tile([128, HL // 2 * L], bf16)      # attn out^T
            wq_a = persist.tile([128, CCH // 2 * 512], bf16)
            wq_b = persist.tile([128, CCH // 2 * 512], bf16)
            wk_sb = persist.tile([128, CCH * 128], bf16)
            wv_sb = persist.tile([128, CCH * 128], bf16)
            wo_sb = persist.tile([128, CCH * 512], bf16)
            pmat = persist.tile([128, 128], bf16)            # rope half-swap
            warm = persist.tile([128, 256], bf16)
            # loop-lifetime pool: released before the o-proj aok pool opens
            loopbuf = tc.alloc_tile_pool(name="loopbuf", bufs=2)
            rope = loopbuf
            c1 = loopbuf.tile([128, L], bf16, tag="c1", bufs=1)
            c2 = loopbuf.tile([128, L], bf16, tag="c2", bufs=1)

            # ---- warmup matmuls: keep the PE busy during initial DMA ----
            nc.gpsimd.memset(warm[:], 0.25)
            for i in range(40):
                psw = pmm.tile([128, 256], f32, tag="mm", name=f"warm{i}")
                nc.tensor.matmul(psw[:], lhsT=warm[:, 0:128], rhs=warm[:],
                                 start=True, stop=True)

            # ---- input DMAs ----
            # sync: wk, wv, wq halves, PM; scalar: x pieces then wo;
            # gpsimd: rope tables.  x is loaded in 4 sub-tiles per token
            # tile so the K-proj chain can start on the first piece.
            nc.sync.dma_start(wk_sb[:], WkP[:])
            nc.sync.dma_start(wv_sb[:], WvP[:])
            nc.sync.dma_start(pmat[:], PM[:])
            WqP_v = WqP[:].rearrange("p (h f) -> p h f", h=2)
            nc.sync.dma_start(wq_a[:], WqP_v[:, 0])
            nc.sync.dma_start(wq_b[:], WqP_v[:, 1])
            xP_v = xP[:].rearrange("p (tt s f) -> p tt s f", tt=TT, s=XSP)
            xts = []  # xts[tt][s] -> [128, 4*512] piece

            def load_x(tt):
                ps = []
                for s in range(XSP):
                    t = rope.tile([128, CCH // XSP * 512], bf16, tag="xt",
                                  bufs=2 * XSP, name=f"xt{tt}_{s}")
                    nc.scalar.dma_start(t[:], xP_v[:, tt, s])
                    ps.append(t)
                xts.append(ps)

            load_x(0)
            nc.gpsimd.dma_start(c1[:], C1[:])
            nc.gpsimd.dma_start(c2[:], C2[:])
            nc.scalar.dma_start(wo_sb[:], WoP[:])

            def xchunk(tt, c):
                return xts[tt][c // XSP][:, (c % XSP) * 512:
                                         (c % XSP + 1) * 512]

            def wqchunk(c, m):
                wt = wq_a if c < CCH // 2 else wq_b
                cc = c % (CCH // 2)
                return wt[:, cc * 512 + m * 128:cc * 512 + (m + 1) * 128]

            # ones blocks of v2 (columns 64:128 and 192:256 of each kt group)
            for off in (64, 192):
                ones_view = bass.AP(v2.tensor, v2.offset + off,
                                    [v2.ap[0], [256, CCH], [1, 64]])
                nc.gpsimd.memset(ones_view, 1.0)

            # causal-mask helpers: ident for the PE transposes, bmask holds
            # -48 where q' < k' within a diagonal 128-block
            ident = persist.tile([128, 128], bf16)
            nc.gpsimd.memset(ident[:], 1.0)
            nc.gpsimd.affine_select(
                out=ident[:], in_=ident[:], pattern=[[-1, 128]],
                compare_op=mybir.AluOpType.is_equal, fill=0.0,
                base=0, channel_multiplier=1)
            bmask = persist.tile([128, 128], bf16)
            nc.gpsimd.memset(bmask[:], -48.0)
            nc.gpsimd.affine_select(
                out=bmask[:], in_=bmask[:],
                pattern=[[-1, 128]], compare_op=mybir.AluOpType.is_gt,
                fill=0.0, base=0, channel_multiplier=1)

            # dummy first collective: the first AG on the CC path costs
            # ~25us extra; absorb it here, overlapped with the projections
            bounce_d = dram.tile([128, 64], bf16, name="bounce_d")
            gath_d = dram.tile([TP * 128, 64], bf16, name="gath_d")
            nc.sync.dma_start(bounce_d[:], warm[:, 0:64])
            nc.gpsimd.collective_compute(
                "AllGather", mybir.AluOpType.bypass,
                replica_groups=[[0, 1, 2, 3], [4, 5, 6, 7]],
                ins=[bounce_d.opt()], outs=[gath_d.opt()])

            def rope_apply(dst_ap, raw_slice, ts, tag):
                """dst = raw*c1 + (P@raw)*c2, swap done on the PE.

                The DVE ops run under high_priority so they sort ahead
                of attention normalizations in the vector FIFO -- roped
                Q/K must trail the projection chains immediately or the
                exp stream stalls at every tile boundary.
                """
                psw = pmm.tile([128, 512], f32, tag="mm", name=f"sw_{tag}")
                nc.tensor.matmul(psw[:], lhsT=pmat[:], rhs=raw_slice,
                                 start=True, stop=True)
                sw = rope.tile([128, 512], bf16, tag="swb",
                               bufs=4, name=f"swb_{tag}")
                with tc.high_priority(offset=900000):
                    nc.vector.tensor_tensor(dst_ap, raw_slice, c1[:, ts],
                                            mybir.AluOpType.mult)
                    nc.vector.tensor_tensor(sw[:], psw[:], c2[:, ts],
                                            mybir.AluOpType.mult)
                    nc.vector.tensor_tensor(dst_ap, dst_ap, sw[:],
                                            mybir.AluOpType.add)

            def proj(tt):
                ts = slice(tt * 512, (tt + 1) * 512)
                if tt + 1 < TT:  # prefetch next tile
                    load_x(tt + 1)

                # --- K, roped per-tile ---
                kraw = rope.tile([128, 512], bf16, tag="kraw")
                psk = pmm.tile([128, 512], f32, tag="mm")
                for c in range(CCH):
                    nc.tensor.matmul(
                        psk[:], lhsT=wk_sb[:, c * 128:(c + 1) * 128],
                        rhs=xchunk(tt, c),
                        start=(c == 0), stop=(c == CCH - 1))
                with tc.high_priority(offset=900000):
                    nc.vector.tensor_copy(kraw[:], psk[:])
                rope_apply(kk[:, ts], kraw[:], ts, f"k{tt}")

                # --- V ---
                psv = pmm.tile([128, 512], f32, tag="mm")
                for c in range(CCH):
                    nc.tensor.matmul(
                        psv[:], lhsT=wv_sb[:, c * 128:(c + 1) * 128],
                        rhs=xchunk(tt, c),
                        start=(c == 0), stop=(c == CCH - 1))
                with tc.high_priority(offset=900000):
                    nc.vector.tensor_copy(v2t[:, ts], psv[:])

                # --- V transpose to token-major via PE transposes ---
                for g in range(2):
                    pst = pmm.tile([128, 256], bf16, tag="mm",
                                   name=f"vt{tt}_{g}")
                    for q in range(4):
                        nc.tensor.transpose(
                            pst[:, q * 64:(q + 1) * 64],
                            v2t[g * 64:(g + 1) * 64,
                                tt * 512 + q * 128:tt * 512 + (q + 1) * 128],
                            ident[g * 64:(g + 1) * 64, g * 64:(g + 1) * 64])
                    v2_dst = bass.AP(v2.tensor,
                                     v2.offset + (4 * tt) * 256 + g * 128,
                                     [v2.ap[0], [256, 4], [1, 64]])
                    with tc.high_priority(offset=900000):
                        nc.vector.tensor_copy(
                            v2_dst, pst[:].rearrange("p (q d) -> p q d",
                                                     d=64))

                # --- Q: 4 M-tiles (head pair (jj, jj+4) each), roped per
                # M-slice so qq trails each chain by ~2.5us ---
                qraw = rope.tile([128, 4 * 512], bf16, tag="qraw")
                for m in range(4):
                    psq = pmm.tile([128, 512], f32, tag="mm")
                    for c in range(CCH):
                        nc.tensor.matmul(
                            psq[:], lhsT=wqchunk(c, m),
                            rhs=xchunk(tt, c),
                            start=(c == 0), stop=(c == CCH - 1))
                    qm = qraw[:, m * 512:(m + 1) * 512]
                    with tc.high_priority(offset=900000):
                        nc.vector.tensor_copy(qm, psq[:])
                    rope_apply(qq[:, m * L + tt * 512:m * L + (tt + 1) * 512],
                               qm, ts, f"q{tt}_{m}")

            def attn_group(jj, qT):
                """Attention for head pair (jj, jj+4), query tile qT."""
                nkt = 4 * qT + 4
                qoff = jj * L
                qs = slice(qoff + qT * 512, qoff + (qT + 1) * 512)
                o0 = po.tile([128, 512], f32, tag="o", name=f"o0_{jj}_{qT}")
                o1 = po.tile([128, 512], f32, tag="o", name=f"o1_{jj}_{qT}")
                for kt in range(nkt):
                    ksl = slice(kt * 128, (kt + 1) * 128)
                    dj = kt - 4 * qT  # >=0 -> diagonal block
                    cut = 128 * dj if dj >= 0 else 0
                    sb = ps_s.tile([128, 1024], f32, tag="s",
                                   name=f"sb_{jj}_{qT}_{kt}")
                    with tc.high_priority(offset=1000000):
                        nc.tensor.matmul(
                            sb[:, cut:512],
                            lhsT=kk[0:64, ksl],
                            rhs=qq[0:64, qs.start + cut:qs.stop], start=True,
                            stop=(dj < 0), tile_position=(0, 0))
                        nc.tensor.matmul(
                            sb[:, 512 + cut:1024],
                            lhsT=kk[64:128, ksl],
                            rhs=qq[64:128, qs.start + cut:qs.stop],
                            start=True,
                            stop=(dj < 0), tile_position=(64, 0))
                        if dj >= 0:
                            nc.tensor.matmul(
                                sb[:, cut:cut + 128],
                                lhsT=ident[:], rhs=bmask[:],
                                start=False, stop=True)
                            nc.tensor.matmul(
                                sb[:, 512 + cut:512 + cut + 128],
                                lhsT=ident[:], rhs=bmask[:],
                                start=False, stop=True)
                    p = pp.tile([128, 1024], bf16, tag="p",
                                name=f"p_{jj}_{qT}_{kt}")
                    sb_v = bass.AP(sb.tensor, sb.offset + cut,
                                   [sb.ap[0], [512, 2], [1, 512 - cut]])
                    p_v = bass.AP(p.tensor, p.offset + cut,
                                  [p.ap[0], [512, 2], [1, 512 - cut]])
                    nc.scalar.activation(
                        p_v, sb_v, mybir.ActivationFunctionType.Exp)
                    pvcut = cut if dj > 0 else 0
                    nc.tensor.matmul(
                        o0[:, pvcut:512],
                        lhsT=v2[:, kt * 256:kt * 256 + 128],
                        rhs=p[:, pvcut:512],
                        start=(kt == 0), stop=(kt == nkt - 1))
                    nc.tensor.matmul(
                        o1[:, pvcut:512],
                        lhsT=v2[:, kt * 256 + 128:(kt + 1) * 256],
                        rhs=p[:, 512 + pvcut:1024],
                        start=(kt == 0), stop=(kt == nkt - 1))
                # normalize (approx-recip full tile; rows 64:128 hold the
                # replicated sums - base!=0 slices break the custom-DVE op)
                rc = pp.tile([128, 512], f32, tag="rc", bufs=2,
                             name=f"rc_{jj}_{qT}")
                nc.vector.reciprocal_approx_fast(rc[:], o0[:])
                nc.vector.tensor_tensor(
                    ao[0:64, qs], o0[0:64, :], rc[64:128, :],
                    mybir.AluOpType.mult)
                rc2 = pp.tile([128, 512], f32, tag="rc", bufs=2,
                              name=f"rc2_{jj}_{qT}")
                nc.vector.reciprocal_approx_fast(rc2[:], o1[:])
                nc.vector.tensor_tensor(
                    ao[64:128, qs], o1[0:64, :], rc2[64:128, :],
                    mybir.AluOpType.mult)

            # AG pieces.  Gathered-chunk layout: wo contraction chunk
            # c = 4*jj + rank; within a chunk, row p = 64*g + d holds
            # q-head (8*rank + jj + 4*g), dim d.
            # Quarters 0/1 ship whole (bounce row = 128*jj + 64*g + d ->
            # gathered block b = 4*rank + jj); quarters 2/3 ship per-jj
            # (bounce row = 64*g + d -> gathered block b = rank).
            bounces = {}
            gaths = {}
            for tq in range(2):
                bounces[tq] = dram.tile([512, 512], bf16, name=f"bounce{tq}")
                gaths[tq] = dram.tile([TP * 512, 512], bf16,
                                      name=f"gath{tq}")
            for tq in (2, 3):
                for jj in range(4):
                    bounces[(tq, jj)] = dram.tile([128, 512], bf16,
                                                  name=f"bounce{tq}{jj}")
                    gaths[(tq, jj)] = dram.tile([TP * 128, 512], bf16,
                                                name=f"gath{tq}{jj}")

            def ship(tq):
                """Bounce ao (all heads, token tile tq) + AllGather."""
                bnc = bounces[tq]
                for g in range(2):
                    for jj in range(4):
                        r0 = 128 * jj + 64 * g
                        nc.sync.dma_start(
                            bnc[r0:r0 + 64, :],
                            ao[g * 64:(g + 1) * 64,
                               jj * L + tq * 512:jj * L + (tq + 1) * 512])
                nc.gpsimd.collective_compute(
                    "AllGather", mybir.AluOpType.bypass,
                    replica_groups=[[0, 1, 2, 3], [4, 5, 6, 7]],
                    ins=[bnc.opt()], outs=[gaths[tq].opt()])

            def shipjj(tq, jj):
                """Head pair (jj, jj+4), token tile tq (tq in {2,3})."""
                bnc = bounces[(tq, jj)]
                for g in range(2):
                    nc.sync.dma_start(
                        bnc[64 * g:64 * g + 64, :],
                        ao[g * 64:(g + 1) * 64,
                           jj * L + tq * 512:jj * L + (tq + 1) * 512])
                nc.gpsimd.collective_compute(
                    "AllGather", mybir.AluOpType.bypass,
                    replica_groups=[[0, 1, 2, 3], [4, 5, 6, 7]],
                    ins=[bnc.opt()], outs=[gaths[(tq, jj)].opt()])

            aoks = {}

            def aok_load(key, nblk, tag, bufs):
                """Load a gathered piece to SBUF (one DMA, gpsimd queue)."""
                aok = aogp.tile([128, nblk * 512], bf16, tag=tag, bufs=bufs,
                                name=f"aok{key}")
                nc.gpsimd.dma_start(
                    aok[:].rearrange("p (b t) -> p b t", t=512),
                    gaths[key][:].rearrange("(b p) t -> p b t", p=128))
                aoks[key] = aok

            def oproj_mms(tq):
                """o-proj for token-quarter tq (chunk c = 4*jj + rank)."""
                ts = slice(tq * 512, (tq + 1) * 512)
                for ct in range(4):
                    pso = pmm.tile([128, 512], f32, tag="mm",
                                   name=f"pso{tq}_{ct}")
                    for jj in range(4):
                        for r in range(TP):
                            c = 4 * jj + r
                            if tq < 2:
                                rhs_t = aoks[tq]
                                b = 4 * r + jj
                            else:
                                rhs_t = aoks[(tq, jj)]
                                b = r
                            nc.tensor.matmul(
                                pso[:],
                                lhsT=wo_sb[:, c * 512 + ct * 128:
                                           c * 512 + (ct + 1) * 128],
                                rhs=rhs_t[:, b * 512:(b + 1) * 512],
                                start=(c == 0), stop=(c == CCH - 1))
                    ost = ostp.tile([128, 512], f32, tag="ost",
                                    name=f"ost{tq}_{ct}")
                    nc.vector.tensor_copy(ost[:], pso[:])
                    nc.gpsimd.dma_start(
                        out[ct * 128:(ct + 1) * 128, ts], ost[:])

            # ================= schedule =================
            for tt in range(TT):
                proj(tt)
                if 1 <= tt <= 2:
                    ship(tt - 1)
                for jj in range(4):
                    attn_group(jj, tt)
                    if tt >= 2:
                        shipjj(tt, jj)
            loopbuf.release()
            aogp = tc.alloc_tile_pool(name="aog", bufs=2)
            aok_load(0, CCH, "aokq", 2)
            oproj_mms(0)
            aok_load(1, CCH, "aokq", 2)
            oproj_mms(1)
            for jj in range(4):
                aok_load((2, jj), TP, "aokp", 8)
            oproj_mms(2)
            for jj in range(4):
                aok_load((3, jj), TP, "aokp", 8)
            oproj_mms(3)
            aogp.release()

    nc.compile()
    return nc


def _host_prep(hidden_states, cos, sin, Wq, Wk, Wv, Wo):
    """Build the 8 per-core input maps (all host-side packing)."""
    scale = float(D) ** -0.5
    # rope coefficient tables [128, L]: 4 groups of 32 rows (d 0:32 pattern)
    cosT = cos[:, :32].T.astype(np.float32)          # [32, L]
    sinT = sin[:, :32].T.astype(np.float32)
    c1 = np.tile(cosT, (4, 1))                       # [128, L]
    c2 = np.concatenate([-sinT, sinT, -sinT, sinT], axis=0)
    # rope half-swap permutation (d -> d XOR 32), as a PE stationary matrix
    pm = np.zeros((128, 128), np.float32)
    pm[np.arange(128) ^ 32, np.arange(128)] = 1.0
    tables = {"C1": c1.astype(BF16), "C2": c2.astype(BF16),
              "PM": pm.astype(BF16)}

    def pack(WT, m):
        # WT [HID, m] -> [128, CCH*m] with row p = concat_c WT[c*128+p, :]
        return np.ascontiguousarray(
            WT.reshape(CCH, 128, m).transpose(1, 0, 2).reshape(128, CCH * m)
        ).astype(BF16)

    # x packed per (tt, c): [128, tt, c, 512]
    xPb = []
    for b in range(B):
        xT = hidden_states[b].T.astype(np.float32)   # [HID, L]
        xp = (xT.reshape(CCH, 128, TT, 512).transpose(1, 2, 0, 3)
              .reshape(128, TT * CCH * 512))
        xPb.append(np.ascontiguousarray(xp).astype(BF16))

    # o-proj contraction-row order: R -> chunk c = R//128 = 4*jj + rank,
    # row p = R%128 = 64*g + d, holding q-head (8*rank + jj + 4*g)
    RR = np.arange(2048)
    perm = ((8 * ((RR // 128) % 4) + RR // 512 + 4 * ((RR % 128) // 64)) * D
            + RR % 64)

    in_maps = []
    for i in range(N_CORES):
        b, r = divmod(i, TP)
        # Wq rows reordered: M-tile m = heads (8r+m, 8r+4+m); scale folded in
        rows = []
        for m in range(4):
            rows.append(Wq[(8 * r + m) * D:(8 * r + m + 1) * D])
            rows.append(Wq[(8 * r + 4 + m) * D:(8 * r + 4 + m + 1) * D])
        WqT_i = (np.concatenate(rows, 0) * scale).T.astype(np.float32)
        WkT_i = Wk[2 * r * D:(2 * r + 2) * D].T.astype(np.float32)
        WvT_i = Wv[2 * r * D:(2 * r + 2) * D].T.astype(np.float32)
        WoT_i = Wo[CW * r:CW * (r + 1), :].T[perm].astype(np.float32)
        in_maps.append({
            "xP": xPb[b],
            "WqP": pack(WqT_i, 512),
            "WkP": pack(WkT_i, 128),
            "WvP": pack(WvT_i, 128),
            "WoP": pack(WoT_i, 512),
            **tables,
        })
    return in_maps


def kernel(hidden_states, cos, sin, Wq, Wk, Wv, Wo, _want_profile=False):
    from concourse.bass_utils import run_bass_kernel_spmd

    if "nc" not in _cache:
        _cache["nc"] = _build_graph()
    nc = _cache["nc"]
    in_maps = _host_prep(np.asarray(hidden_states), np.asarray(cos),
                         np.asarray(sin), np.asarray(Wq), np.asarray(Wk),
                         np.asarray(Wv), np.asarray(Wo))
    res = run_bass_kernel_spmd(nc, in_maps, list(range(N_CORES)),
                               trace=_want_profile)
    # assemble: core (b, r) holds out^T [512, L] = cols [512r, 512r+512) of b
    full = np.empty((B, L, HID), np.float32)
    for i in range(N_CORES):
        b, r = divmod(i, TP)
        full[b, :, CW * r:CW * (r + 1)] = res.results[i]["out"].T
    if _want_profile:
        return full, res
    return full


# revision 24
# speedup vs baseline: 1.0107x; 1.0107x over previous
"""Distributed GQA attention kernel for one TRN2 chip (8 NeuronCores).

Problem: B=2, L=2048, HID=2048, H=32 q-heads, HKV=8 kv-heads, D=64,
rotary embedding, causal softmax, o-proj.

Sharding: core i -> batch b=i//4, TP rank r=i%4.  Each core computes
8 q-heads / 2 kv-heads of its batch, all-gathers the attention outputs
(feature-major, bf16) within its 4-core TP group, then computes its
512 output columns of the o-proj.  Host assembles the full output.

Schedule (v4):
- proj(tt) and attention interleave per q-tile so the scalar-engine exp
  stream overlaps projection matmuls.
- Per key-tile kt, BOTH heads' scores go into ONE [128,1024] PSUM tile
  so the two K=64 quadrant matmuls share one slot-alloc event and issue
  back-to-back under tc.high_priority -> they run CONCURRENTLY on
  disjoint PE row halves.  One exp per kt covers both heads, restricted
  to the causally-valid column range via a strided AP.
- RoPE's partition half-swap runs on the PE (constant permutation
  matrix as stationary operand) instead of DMA, per M-slice, so roped
  Q/K trail the projection chains by ~2.5us instead of ~9us.
- AllGathers: quarters 0/1 ship whole (after proj(1)/proj(2)); quarters
  2/3 ship per-head-pair as each attention group completes, so o-proj
  chunks stream in arrival order and only the last small piece's
  16 matmuls remain after the final AG.
- Queues: sync = weights + ship bounces; scalar = x tiles + Wo;
  gpsimd = rope tables, gather loads, output stores (no head-blocking).
- x arrives as 4 sub-tiles per token tile and Wq as 2 halves so the
  first K-proj matmul can start at ~18us.
- All matmuls bf16 with fp32 PSUM.  Softmax skips the row-max (logits
  bounded ~|6|) and gets row sums free from a 64-wide ones block in V's
  stationary operand; normalization is a DVE reciprocal + multiply.
"""

import sys

sys.path.insert(0, "/opt/trn_rl_repo")

import numpy as np
import ml_dtypes

B, L, HID = 2, 2048, 2048
H, HKV, D = 32, 8, 64
N_CORES = 8
TP = 4           # tensor-parallel group size
HL = 8           # q heads per core
CW = 512         # o-proj output columns per core
TT = 4           # t tiles of 512 over L
CCH = HID // 128 # contraction chunks (16)
XSP = 4          # x sub-tiles per token tile
BF16 = ml_dtypes.bfloat16

_cache = {}


def _build_graph():
    import concourse.bass as bass
    import concourse.tile as tile
    from concourse import bacc, mybir

    dt = mybir.dt
    f32, bf16 = dt.float32, dt.bfloat16

    nc = bacc.Bacc("TRN2", target_bir_lowering=False, debug=False,
                   num_devices=N_CORES)

    # all packed [128, n] with per-partition-contiguous layout
    xP = nc.dram_tensor("xP", [128, TT * CCH * 512], bf16,
                        kind="ExternalInput")
    WqP = nc.dram_tensor("WqP", [128, CCH * 512], bf16, kind="ExternalInput")
    WkP = nc.dram_tensor("WkP", [128, CCH * 128], bf16, kind="ExternalInput")
    WvP = nc.dram_tensor("WvP", [128, CCH * 128], bf16, kind="ExternalInput")
    WoP = nc.dram_tensor("WoP", [128, CCH * 512], bf16, kind="ExternalInput")
    C1 = nc.dram_tensor("C1", [128, L], bf16, kind="ExternalInput")
    C2 = nc.dram_tensor("C2", [128, L], bf16, kind="ExternalInput")
    PM = nc.dram_tensor("PM", [128, 128], bf16, kind="ExternalInput")
    out = nc.dram_tensor("out", [CW, L], f32, kind="ExternalOutput")

    with tile.TileContext(nc) as tc:
        with (
            tc.tile_pool(name="persist", bufs=1) as persist,
            tc.tile_pool(name="mm", bufs=2, space="PSUM") as pmm,
            tc.tile_pool(name="ps_s", bufs=2, space="PSUM") as ps_s,
            tc.tile_pool(name="po", bufs=2, space="PSUM") as po,
            tc.tile_pool(name="pp", bufs=4) as pp,
            tc.tile_pool(name="ost", bufs=2) as ostp,
            tc.tile_pool(name="dram", bufs=1, space="DRAM") as dram,
        ):
            # ---- persistent SBUF tensors ----
            qq = persist.tile([128, HL // 2 * L], bf16)      # roped Q^T, 2MB
            kk = persist.tile([128, L], bf16)                # roped K^T (2 kv)
            v2t = persist.tile([128, L], bf16)               # V^T staging
            v2 = persist.tile([128, CCH * 256], bf16)        # [V|1|V|1] per kt
            ao = persist.tile([128, HL // 2 * L], bf16)      # attn out^T
            wq_a = persist.tile([128, CCH // 2 * 512], bf16)
            wq_b = persist.tile([128, CCH // 2 * 512], bf16)
            wk_sb = persist.tile([128, CCH * 128], bf16)
            wv_sb = persist.tile([128, CCH * 128], bf16)
            wo_sb = persist.tile([128, CCH * 512], bf16)
            pmat = persist.tile([128, 128], bf16)            # rope half-swap
            warm = persist.tile([128, 256], bf16)
            # loop-lifetime pool: released before the o-proj aok pool opens
            loopbuf = tc.alloc_tile_pool(name="loopbuf", bufs=2)
            rope = loopbuf
            c1 = loopbuf.tile([128, L], bf16, tag="c1", bufs=1)
            c2 = loopbuf.tile([128, L], bf16, tag="c2", bufs=1)

            # ---- warmup matmuls: keep the PE busy during initial DMA ----
            nc.gpsimd.memset(warm[:], 0.25)
            for i in range(40):
                psw = pmm.tile([128, 256], f32, tag="mm", name=f"warm{i}")
                nc.tensor.matmul(psw[:], lhsT=warm[:, 0:128], rhs=warm[:],
                                 start=True, stop=True)

            # ---- input DMAs ----
            # sync: wk, wv, wq halves, PM; scalar: x pieces then wo;
            # gpsimd: rope tables.  x is loaded in 4 sub-tiles per token
            # tile so the K-proj chain can start on the first piece.
            nc.sync.dma_start(wk_sb[:], WkP[:])
            nc.sync.dma_start(wv_sb[:], WvP[:])
            nc.sync.dma_start(pmat[:], PM[:])
            WqP_v = WqP[:].rearrange("p (h f) -> p h f", h=2)
            nc.sync.dma_start(wq_a[:], WqP_v[:, 0])
            nc.sync.dma_start(wq_b[:], WqP_v[:, 1])
            xP_v = xP[:].rearrange("p (tt s f) -> p tt s f", tt=TT, s=XSP)
            xts = []  # xts[tt][s] -> [128, 4*512] piece

            def load_x(tt):
                ps = []
                for s in range(XSP):
                    t = rope.tile([128, CCH // XSP * 512], bf16, tag="xt",
                                  bufs=2 * XSP, name=f"xt{tt}_{s}")
                    nc.scalar.dma_start(t[:], xP_v[:, tt, s])
                    ps.append(t)
                xts.append(ps)

            load_x(0)
            nc.gpsimd.dma_start(c1[:], C1[:])
            nc.gpsimd.dma_start(c2[:], C2[:])
            nc.scalar.dma_start(wo_sb[:], WoP[:])

            def xchunk(tt, c):
                return xts[tt][c // XSP][:, (c % XSP) * 512:
                                         (c % XSP + 1) * 512]

            def wqchunk(c, m):
                wt = wq_a if c < CCH // 2 else wq_b
                cc = c % (CCH // 2)
                return wt[:, cc * 512 + m * 128:cc * 512 + (m + 1) * 128]

            # ones blocks of v2 (columns 64:128 and 192:256 of each kt group)
            for off in (64, 192):
                ones_view = bass.AP(v2.tensor, v2.offset + off,
                                    [v2.ap[0], [256, CCH], [1, 64]])
                nc.gpsimd.memset(ones_view, 1.0)

            # causal-mask helpers: ident for the PE transposes, bmask holds
            # -48 where q' < k' within a diagonal 128-block
            ident = persist.tile([128, 128], bf16)
            nc.gpsimd.memset(ident[:], 1.0)
            nc.gpsimd.affine_select(
                out=ident[:], in_=ident[:], pattern=[[-1, 128]],
                compare_op=mybir.AluOpType.is_equal, fill=0.0,
                base=0, channel_multiplier=1)
            bmask = persist.tile([128, 128], bf16)
            nc.gpsimd.memset(bmask[:], -48.0)
            nc.gpsimd.affine_select(
                out=bmask[:], in_=bmask[:],
                pattern=[[-1, 128]], compare_op=mybir.AluOpType.is_gt,
                fill=0.0, base=0, channel_multiplier=1)

            # dummy first collective: the first AG on the CC path costs
            # ~25us extra; absorb it here, overlapped with the projections
            bounce_d = dram.tile([128, 64], bf16, name="bounce_d")
            gath_d = dram.tile([TP * 128, 64], bf16, name="gath_d")
            nc.sync.dma_start(bounce_d[:], warm[:, 0:64])
            nc.gpsimd.collective_compute(
                "AllGather", mybir.AluOpType.bypass,
                replica_groups=[[0, 1, 2, 3], [4, 5, 6, 7]],
                ins=[bounce_d.opt()], outs=[gath_d.opt()])

            def rope_apply(dst_ap, raw_slice, ts, tag):
                """dst = raw*c1 + (P@raw)*c2, swap done on the PE.

                The DVE ops run under high_priority so they sort ahead
                of attention normalizations in the vector FIFO -- roped
                Q/K must trail the projection chains immediately or the
                exp stream stalls at every tile boundary.
                """
                psw = pmm.tile([128, 512], f32, tag="mm", name=f"sw_{tag}")
                nc.tensor.matmul(psw[:], lhsT=pmat[:], rhs=raw_slice,
                                 start=True, stop=True)
                sw = rope.tile([128, 512], bf16, tag="swb",
                               bufs=4, name=f"swb_{tag}")
                nc.vector.tensor_tensor(dst_ap, raw_slice, c1[:, ts],
                                        mybir.AluOpType.mult)
                nc.vector.tensor_tensor(sw[:], psw[:], c2[:, ts],
                                        mybir.AluOpType.mult)
                nc.vector.tensor_tensor(dst_ap, dst_ap, sw[:],
                                        mybir.AluOpType.add)

            def proj_k(tt, ts):
                kraw = rope.tile([128, 512], bf16, tag="kraw")
                psk = pmm.tile([128, 512], f32, tag="mm")
                for c in range(CCH):
                    nc.tensor.matmul(
                        psk[:], lhsT=wk_sb[:, c * 128:(c + 1) * 128],
                        rhs=xchunk(tt, c),
                        start=(c == 0), stop=(c == CCH - 1))
                nc.vector.tensor_copy(kraw[:], psk[:])
                rope_apply(kk[:, ts], kraw[:], ts, f"k{tt}")

            def proj_v(tt, ts):
                psv = pmm.tile([128, 512], f32, tag="mm")
                for c in range(CCH):
                    nc.tensor.matmul(
                        psv[:], lhsT=wv_sb[:, c * 128:(c + 1) * 128],
                        rhs=xchunk(tt, c),
                        start=(c == 0), stop=(c == CCH - 1))
                nc.vector.tensor_copy(v2t[:, ts], psv[:])
                # V transpose to token-major via PE transposes (DMA
                # transposes would serialize against collective SDMA)
                for g in range(2):
                    pst = pmm.tile([128, 256], bf16, tag="mm",
                                   name=f"vt{tt}_{g}")
                    for q in range(4):
                        nc.tensor.transpose(
                            pst[:, q * 64:(q + 1) * 64],
                            v2t[g * 64:(g + 1) * 64,
                                tt * 512 + q * 128:tt * 512 + (q + 1) * 128],
                            ident[g * 64:(g + 1) * 64, g * 64:(g + 1) * 64])
                    v2_dst = bass.AP(v2.tensor,
                                     v2.offset + (4 * tt) * 256 + g * 128,
                                     [v2.ap[0], [256, 4], [1, 64]])
                    nc.vector.tensor_copy(
                        v2_dst, pst[:].rearrange("p (q d) -> p q d", d=64))

            def proj_q(tt, ts, qraw, m):
                psq = pmm.tile([128, 512], f32, tag="mm")
                for c in range(CCH):
                    nc.tensor.matmul(
                        psq[:], lhsT=wqchunk(c, m),
                        rhs=xchunk(tt, c),
                        start=(c == 0), stop=(c == CCH - 1))
                qm = qraw[:, m * 512:(m + 1) * 512]
                nc.vector.tensor_copy(qm, psq[:])
                rope_apply(qq[:, m * L + tt * 512:m * L + (tt + 1) * 512],
                           qm, ts, f"q{tt}_{m}")

            def proj(tt):
                ts = slice(tt * 512, (tt + 1) * 512)
                if tt + 1 < TT:  # prefetch next tile
                    load_x(tt + 1)
                qraw = rope.tile([128, 4 * 512], bf16, tag="qraw")
                if tt == 0:
                    # all of attn(qT=0) is diagonal: K/V first
                    proj_k(tt, ts)
                    proj_v(tt, ts)
                    for m in range(4):
                        proj_q(tt, ts, qraw, m)
                else:
                    # attn(jj=0, qT=tt)'s early key-tiles need only
                    # qq(m=0) + old kk, so the exp stream restarts after
                    # just Qm0+K; V/VT trail (PV of the diagonal kts
                    # lags, covered by deep p bufs)
                    proj_q(tt, ts, qraw, 0)
                    proj_k(tt, ts)
                    for m in range(1, 4):
                        proj_q(tt, ts, qraw, m)
                    proj_v(tt, ts)

            def attn_group(jj, qT):
                """Attention for head pair (jj, jj+4), query tile qT."""
                nkt = 4 * qT + 4
                qoff = jj * L
                qs = slice(qoff + qT * 512, qoff + (qT + 1) * 512)
                o0 = po.tile([128, 512], f32, tag="o", name=f"o0_{jj}_{qT}")
                o1 = po.tile([128, 512], f32, tag="o", name=f"o1_{jj}_{qT}")
                for kt in range(nkt):
                    ksl = slice(kt * 128, (kt + 1) * 128)
                    dj = kt - 4 * qT  # >=0 -> diagonal block
                    cut = 128 * dj if dj >= 0 else 0
                    sb = ps_s.tile([128, 1024], f32, tag="s",
                                   name=f"sb_{jj}_{qT}_{kt}")
                    with tc.high_priority(offset=1000000):
                        nc.tensor.matmul(
                            sb[:, cut:512],
                            lhsT=kk[0:64, ksl],
                            rhs=qq[0:64, qs.start + cut:qs.stop], start=True,
                            stop=(dj < 0), tile_position=(0, 0))
                        nc.tensor.matmul(
                            sb[:, 512 + cut:1024],
                            lhsT=kk[64:128, ksl],
                            rhs=qq[64:128, qs.start + cut:qs.stop],
                            start=True,
                            stop=(dj < 0), tile_position=(64, 0))
                        if dj >= 0:
                            nc.tensor.matmul(
                                sb[:, cut:cut + 128],
                                lhsT=ident[:], rhs=bmask[:],
                                start=False, stop=True)
                            nc.tensor.matmul(
                                sb[:, 512 + cut:512 + cut + 128],
                                lhsT=ident[:], rhs=bmask[:],
                                start=False, stop=True)
                    p = pp.tile([128, 1024], bf16, tag="p", bufs=10,
                                name=f"p_{jj}_{qT}_{kt}")
                    sb_v = bass.AP(sb.tensor, sb.offset + cut,
                                   [sb.ap[0], [512, 2], [1, 512 - cut]])
                    p_v = bass.AP(p.tensor, p.offset + cut,
                                  [p.ap[0], [512, 2], [1, 512 - cut]])
                    nc.scalar.activation(
                        p_v, sb_v, mybir.ActivationFunctionType.Exp)
                    pvcut = cut if dj > 0 else 0
                    nc.tensor.matmul(
                        o0[:, pvcut:512],
                        lhsT=v2[:, kt * 256:kt * 256 + 128],
                        rhs=p[:, pvcut:512],
                        start=(kt == 0), stop=(kt == nkt - 1))
                    nc.tensor.matmul(
                        o1[:, pvcut:512],
                        lhsT=v2[:, kt * 256 + 128:(kt + 1) * 256],
                        rhs=p[:, 512 + pvcut:1024],
                        start=(kt == 0), stop=(kt == nkt - 1))
                # normalize (approx-recip full tile; rows 64:128 hold the
                # replicated sums - base!=0 slices break the custom-DVE op)
                rc = pp.tile([128, 512], f32, tag="rc", bufs=2,
                             name=f"rc_{jj}_{qT}")
                nc.vector.reciprocal_approx_fast(rc[:], o0[:])
                nc.vector.tensor_tensor(
                    ao[0:64, qs], o0[0:64, :], rc[64:128, :],
                    mybir.AluOpType.mult)
                rc2 = pp.tile([128, 512], f32, tag="rc", bufs=2,
                              name=f"rc2_{jj}_{qT}")
                nc.vector.reciprocal_approx_fast(rc2[:], o1[:])
                nc.vector.tensor_tensor(
                    ao[64:128, qs], o1[0:64, :], rc2[64:128, :],
                    mybir.AluOpType.mult)

            # AG pieces.  Gathered-chunk layout: wo contraction chunk
            # c = 4*jj + rank; within a chunk, row p = 64*g + d holds
            # q-head (8*rank + jj + 4*g), dim d.  All quarters ship whole
            # (bounce row = 128*jj + 64*g + d -> gathered block
            # b = 4*rank + jj): few big AGs keep the serial CC core -- in
            # both the scheduler's cost model and reality -- short.
            bounces = {}
            gaths = {}
            for tq in range(TT):
                bounces[tq] = dram.tile([512, 512], bf16, name=f"bounce{tq}")
                gaths[tq] = dram.tile([TP * 512, 512], bf16,
                                      name=f"gath{tq}")

            def ship(tq):
                """Bounce ao (all heads, token tile tq) + AllGather."""
                bnc = bounces[tq]
                for g in range(2):
                    for jj in range(4):
                        r0 = 128 * jj + 64 * g
                        nc.sync.dma_start(
                            bnc[r0:r0 + 64, :],
                            ao[g * 64:(g + 1) * 64,
                               jj * L + tq * 512:jj * L + (tq + 1) * 512])
                nc.gpsimd.collective_compute(
                    "AllGather", mybir.AluOpType.bypass,
                    replica_groups=[[0, 1, 2, 3], [4, 5, 6, 7]],
                    ins=[bnc.opt()], outs=[gaths[tq].opt()])

            aoks = {}

            def aok_load(key, nblk, tag, bufs):
                """Load a gathered piece to SBUF (one DMA, gpsimd queue)."""
                aok = aogp.tile([128, nblk * 512], bf16, tag=tag, bufs=bufs,
                                name=f"aok{key}")
                nc.gpsimd.dma_start(
                    aok[:].rearrange("p (b t) -> p b t", t=512),
                    gaths[key][:].rearrange("(b p) t -> p b t", p=128))
                aoks[key] = aok

            def oproj_mms(tq):
                """o-proj for token-quarter tq (chunk c = 4*jj + rank)."""
                ts = slice(tq * 512, (tq + 1) * 512)
                for ct in range(4):
                    pso = pmm.tile([128, 512], f32, tag="mm",
                                   name=f"pso{tq}_{ct}")
                    for jj in range(4):
                        for r in range(TP):
                            c = 4 * jj + r
                            b = 4 * r + jj
                            nc.tensor.matmul(
                                pso[:],
                                lhsT=wo_sb[:, c * 512 + ct * 128:
                                           c * 512 + (ct + 1) * 128],
                                rhs=aoks[tq][:, b * 512:(b + 1) * 512],
                                start=(c == 0), stop=(c == CCH - 1))
                    ost = ostp.tile([128, 512], f32, tag="ost",
                                    name=f"ost{tq}_{ct}")
                    nc.vector.tensor_copy(ost[:], pso[:])
                    nc.gpsimd.dma_start(
                        out[ct * 128:(ct + 1) * 128, ts], ost[:])

            # ================= schedule =================
            for tt in range(TT):
                proj(tt)
                if tt >= 1:
                    ship(tt - 1)
                for jj in range(4):
                    attn_group(jj, tt)
            ship(3)
            loopbuf.release()
            aogp = tc.alloc_tile_pool(name="aog", bufs=3)
            for tq in range(TT):
                aok_load(tq, CCH, "aokq", 3)
                oproj_mms(tq)
            aogp.release()

    nc.compile()
    return nc


def _host_prep(hidden_states, cos, sin, Wq, Wk, Wv, Wo):
    """Build the 8 per-core input maps (all host-side packing)."""
    scale = float(D) ** -0.5
    # rope coefficient tables [128, L]: 4 groups of 32 rows (d 0:32 pattern)
    cosT = cos[:, :32].T.astype(np.float32)          # [32, L]
    sinT = sin[:, :32].T.astype(np.float32)
    c1 = np.tile(cosT, (4, 1))                       # [128, L]
    c2 = np.concatenate([-sinT, sinT, -sinT, sinT], axis=0)
    # rope half-swap permutation (d -> d XOR 32), as a PE stationary matrix
    pm = np.zeros((128, 128), np.float32)
    pm[np.arange(128) ^ 32, np.arange(128)] = 1.0
    tables = {"C1": c1.astype(BF16), "C2": c2.astype(BF16),
              "PM": pm.astype(BF16)}

    def pack(WT, m):
        # WT [HID, m] -> [128, CCH*m] with row p = concat_c WT[c*128+p, :]
        return np.ascontiguousarray(
            WT.reshape(CCH, 128, m).transpose(1, 0, 2).reshape(128, CCH * m)
        ).astype(BF16)

    # x packed per (tt, c): [128, tt, c, 512]
    xPb = []
    for b in range(B):
        xT = hidden_states[b].T.astype(np.float32)   # [HID, L]
        xp = (xT.reshape(CCH, 128, TT, 512).transpose(1, 2, 0, 3)
              .reshape(128, TT * CCH * 512))
        xPb.append(np.ascontiguousarray(xp).astype(BF16))

    # o-proj contraction-row order: R -> chunk c = R//128 = 4*jj + rank,
    # row p = R%128 = 64*g + d, holding q-head (8*rank + jj + 4*g)
    RR = np.arange(2048)
    perm = ((8 * ((RR // 128) % 4) + RR // 512 + 4 * ((RR % 128) // 64)) * D
            + RR % 64)

    in_maps = []
    for i in range(N_CORES):
        b, r = divmod(i, TP)
        # Wq rows reordered: M-tile m = heads (8r+m, 8r+4+m); scale folded in
        rows = []
        for m in range(4):
            rows.append(Wq[(8 * r + m) * D:(8 * r + m + 1) * D])
            rows.append(Wq[(8 * r + 4 + m) * D:(8 * r + 4 + m + 1) * D])
        WqT_i = (np.concatenate(rows, 0) * scale).T.astype(np.float32)
        WkT_i = Wk[2 * r * D:(2 * r + 2) * D].T.astype(np.float32)
        WvT_i = Wv[2 * r * D:(2 * r + 2) * D].T.astype(np.float32)
        WoT_i = Wo[CW * r:CW * (r + 1), :].T[perm].astype(np.float32)
        in_maps.append({
            "xP": xPb[b],
            "WqP": pack(WqT_i, 512),
            "WkP": pack(WkT_i, 128),
            "WvP": pack(WvT_i, 128),
            "WoP": pack(WoT_i, 512),
            **tables,
        })
    return in_maps


def kernel(hidden_states, cos, sin, Wq, Wk, Wv, Wo, _want_profile=False):
    from concourse.bass_utils import run_bass_kernel_spmd

    if "nc" not in _cache:
        _cache["nc"] = _build_graph()
    nc = _cache["nc"]
    in_maps = _host_prep(np.asarray(hidden_states), np.asarray(cos),
                         np.asarray(sin), np.asarray(Wq), np.asarray(Wk),
                         np.asarray(Wv), np.asarray(Wo))
    res = run_bass_kernel_spmd(nc, in_maps, list(range(N_CORES)),
                               trace=_want_profile)
    # assemble: core (b, r) holds out^T [512, L] = cols [512r, 512r+512) of b
    full = np.empty((B, L, HID), np.float32)
    for i in range(N_CORES):
        b, r = divmod(i, TP)
        full[b, :, CW * r:CW * (r + 1)] = res.results[i]["out"].T
    if _want_profile:
        return full, res
    return full


# revision 25
# speedup vs baseline: 1.0673x; 1.0560x over previous
"""Distributed GQA attention kernel for one TRN2 chip (8 NeuronCores).

Problem: B=2, L=2048, HID=2048, H=32 q-heads, HKV=8 kv-heads, D=64,
rotary embedding, causal softmax, o-proj.

Sharding: core i -> batch b=i//4, TP rank r=i%4.  Each core computes
8 q-heads / 2 kv-heads of its batch, all-gathers the attention outputs
(feature-major, bf16) within its 4-core TP group, then computes its
512 output columns of the o-proj.  Host assembles the full output.

Schedule (v4):
- proj(tt) and attention interleave per q-tile so the scalar-engine exp
  stream overlaps projection matmuls.
- Per key-tile kt, BOTH heads' scores go into ONE [128,1024] PSUM tile
  so the two K=64 quadrant matmuls share one slot-alloc event and issue
  back-to-back under tc.high_priority -> they run CONCURRENTLY on
  disjoint PE row halves.  One exp per kt covers both heads, restricted
  to the causally-valid column range via a strided AP.
- RoPE's partition half-swap runs on the PE (constant permutation
  matrix as stationary operand) instead of DMA, per M-slice, so roped
  Q/K trail the projection chains by ~2.5us instead of ~9us.
- AllGathers: quarters 0/1 ship whole (after proj(1)/proj(2)); quarters
  2/3 ship per-head-pair as each attention group completes, so o-proj
  chunks stream in arrival order and only the last small piece's
  16 matmuls remain after the final AG.
- Queues: sync = weights + ship bounces; scalar = x tiles + Wo;
  gpsimd = rope tables, gather loads, output stores (no head-blocking).
- x arrives as 4 sub-tiles per token tile and Wq as 2 halves so the
  first K-proj matmul can start at ~18us.
- All matmuls bf16 with fp32 PSUM.  Softmax skips the row-max (logits
  bounded ~|6|) and gets row sums free from a 64-wide ones block in V's
  stationary operand; normalization is a DVE reciprocal + multiply.
"""

import sys

sys.path.insert(0, "/opt/trn_rl_repo")

import numpy as np
import ml_dtypes

B, L, HID = 2, 2048, 2048
H, HKV, D = 32, 8, 64
N_CORES = 8
TP = 4           # tensor-parallel group size
HL = 8           # q heads per core
CW = 512         # o-proj output columns per core
TT = 4           # t tiles of 512 over L
CCH = HID // 128 # contraction chunks (16)
XSP = 4          # x sub-tiles per token tile
BF16 = ml_dtypes.bfloat16

_cache = {}


def _build_graph():
    import concourse.bass as bass
    import concourse.tile as tile
    from concourse import bacc, mybir

    dt = mybir.dt
    f32, bf16 = dt.float32, dt.bfloat16

    nc = bacc.Bacc("TRN2", target_bir_lowering=False, debug=False,
                   num_devices=N_CORES)

    # all packed [128, n] with per-partition-contiguous layout
    xP = nc.dram_tensor("xP", [128, TT * CCH * 512], bf16,
                        kind="ExternalInput")
    WqP = nc.dram_tensor("WqP", [128, CCH * 512], bf16, kind="ExternalInput")
    WkP = nc.dram_tensor("WkP", [128, CCH * 128], bf16, kind="ExternalInput")
    WvP = nc.dram_tensor("WvP", [128, CCH * 128], bf16, kind="ExternalInput")
    WoP = nc.dram_tensor("WoP", [128, CCH * 512], bf16, kind="ExternalInput")
    C1 = nc.dram_tensor("C1", [128, L], bf16, kind="ExternalInput")
    C2 = nc.dram_tensor("C2", [128, L], bf16, kind="ExternalInput")
    PM = nc.dram_tensor("PM", [128, 128], bf16, kind="ExternalInput")
    out = nc.dram_tensor("out", [CW, L], f32, kind="ExternalOutput")

    with tile.TileContext(nc) as tc:
        with (
            tc.tile_pool(name="persist", bufs=1) as persist,
            tc.tile_pool(name="mm", bufs=2, space="PSUM") as pmm,
            tc.tile_pool(name="ps_s", bufs=2, space="PSUM") as ps_s,
            tc.tile_pool(name="po", bufs=2, space="PSUM") as po,
            tc.tile_pool(name="pp", bufs=4) as pp,
            tc.tile_pool(name="ost", bufs=2) as ostp,
            tc.tile_pool(name="dram", bufs=1, space="DRAM") as dram,
        ):
            # ---- persistent SBUF tensors ----
            qq = persist.tile([128, HL // 2 * L], bf16)      # roped Q^T, 2MB
            kk = persist.tile([128, L], bf16)                # roped K^T (2 kv)
            v2t = persist.tile([128, L], bf16)               # V^T staging
            v2 = persist.tile([128, CCH * 256], bf16)        # [V|1|V|1] per kt
            ao = persist.tile([128, HL // 2 * L], bf16)      # attn out^T
            wq_a = persist.tile([128, CCH // 2 * 512], bf16)
            wq_b = persist.tile([128, CCH // 2 * 512], bf16)
            wk_sb = persist.tile([128, CCH * 128], bf16)
            wv_sb = persist.tile([128, CCH * 128], bf16)
            wo_sb = persist.tile([128, CCH * 512], bf16)
            pmat = persist.tile([128, 128], bf16)            # rope half-swap
            warm = persist.tile([128, 256], bf16)
            # loop-lifetime pool: released before the o-proj aok pool opens
            loopbuf = tc.alloc_tile_pool(name="loopbuf", bufs=2)
            rope = loopbuf
            c1 = loopbuf.tile([128, L], bf16, tag="c1", bufs=1)
            c2 = loopbuf.tile([128, L], bf16, tag="c2", bufs=1)

            # ---- warmup matmuls: keep the PE busy during initial DMA ----
            nc.gpsimd.memset(warm[:], 0.25)
            for i in range(40):
                psw = pmm.tile([128, 256], f32, tag="mm", name=f"warm{i}")
                nc.tensor.matmul(psw[:], lhsT=warm[:, 0:128], rhs=warm[:],
                                 start=True, stop=True)

            # ---- input DMAs ----
            # sync: wk, wv, wq halves, PM; scalar: x pieces then wo;
            # gpsimd: rope tables.  x is loaded in 4 sub-tiles per token
            # tile so the K-proj chain can start on the first piece.
            nc.sync.dma_start(wk_sb[:], WkP[:])
            nc.sync.dma_start(wv_sb[:], WvP[:])
            nc.sync.dma_start(pmat[:], PM[:])
            WqP_v = WqP[:].rearrange("p (h f) -> p h f", h=2)
            nc.sync.dma_start(wq_a[:], WqP_v[:, 0])
            nc.sync.dma_start(wq_b[:], WqP_v[:, 1])
            xP_v = xP[:].rearrange("p (tt s f) -> p tt s f", tt=TT, s=XSP)
            xts = []  # xts[tt][s] -> [128, 4*512] piece

            def load_x(tt):
                ps = []
                for s in range(XSP):
                    t = rope.tile([128, CCH // XSP * 512], bf16, tag="xt",
                                  bufs=2 * XSP, name=f"xt{tt}_{s}")
                    nc.scalar.dma_start(t[:], xP_v[:, tt, s])
                    ps.append(t)
                xts.append(ps)

            load_x(0)
            nc.gpsimd.dma_start(c1[:], C1[:])
            nc.gpsimd.dma_start(c2[:], C2[:])
            nc.scalar.dma_start(wo_sb[:], WoP[:])

            def xchunk(tt, c):
                return xts[tt][c // XSP][:, (c % XSP) * 512:
                                         (c % XSP + 1) * 512]

            def wqchunk(c, m):
                wt = wq_a if c < CCH // 2 else wq_b
                cc = c % (CCH // 2)
                return wt[:, cc * 512 + m * 128:cc * 512 + (m + 1) * 128]

            # ones blocks of v2 (columns 64:128 and 192:256 of each kt group)
            for off in (64, 192):
                ones_view = bass.AP(v2.tensor, v2.offset + off,
                                    [v2.ap[0], [256, CCH], [1, 64]])
                nc.gpsimd.memset(ones_view, 1.0)

            # causal-mask helpers: ident for the PE transposes, bmask holds
            # -48 where q' < k' within a diagonal 128-block
            ident = persist.tile([128, 128], bf16)
            nc.gpsimd.memset(ident[:], 1.0)
            nc.gpsimd.affine_select(
                out=ident[:], in_=ident[:], pattern=[[-1, 128]],
                compare_op=mybir.AluOpType.is_equal, fill=0.0,
                base=0, channel_multiplier=1)
            bmask = persist.tile([128, 128], bf16)
            nc.gpsimd.memset(bmask[:], -48.0)
            nc.gpsimd.affine_select(
                out=bmask[:], in_=bmask[:],
                pattern=[[-1, 128]], compare_op=mybir.AluOpType.is_gt,
                fill=0.0, base=0, channel_multiplier=1)

            # dummy first collective: the first AG on the CC path costs
            # ~25us extra; absorb it here, overlapped with the projections
            bounce_d = dram.tile([128, 64], bf16, name="bounce_d")
            gath_d = dram.tile([TP * 128, 64], bf16, name="gath_d")
            nc.sync.dma_start(bounce_d[:], warm[:, 0:64])
            nc.gpsimd.collective_compute(
                "AllGather", mybir.AluOpType.bypass,
                replica_groups=[[0, 1, 2, 3], [4, 5, 6, 7]],
                ins=[bounce_d.opt()], outs=[gath_d.opt()])

            def rope_apply(dst_ap, raw_slice, ts, tag):
                """dst = raw*c1 + (P@raw)*c2, swap done on the PE.

                The DVE ops run under high_priority so they sort ahead
                of attention normalizations in the vector FIFO -- roped
                Q/K must trail the projection chains immediately or the
                exp stream stalls at every tile boundary.
                """
                psw = pmm.tile([128, 512], f32, tag="mm", name=f"sw_{tag}")
                nc.tensor.matmul(psw[:], lhsT=pmat[:], rhs=raw_slice,
                                 start=True, stop=True)
                sw = rope.tile([128, 512], bf16, tag="swb",
                               bufs=4, name=f"swb_{tag}")
                nc.vector.tensor_tensor(dst_ap, raw_slice, c1[:, ts],
                                        mybir.AluOpType.mult)
                nc.vector.tensor_tensor(sw[:], psw[:], c2[:, ts],
                                        mybir.AluOpType.mult)
                nc.vector.tensor_tensor(dst_ap, dst_ap, sw[:],
                                        mybir.AluOpType.add)

            def proj_k(tt, ts):
                kraw = rope.tile([128, 512], bf16, tag="kraw")
                psk = pmm.tile([128, 512], f32, tag="mm")
                for c in range(CCH):
                    nc.tensor.matmul(
                        psk[:], lhsT=wk_sb[:, c * 128:(c + 1) * 128],
                        rhs=xchunk(tt, c),
                        start=(c == 0), stop=(c == CCH - 1))
                nc.vector.tensor_copy(kraw[:], psk[:])
                rope_apply(kk[:, ts], kraw[:], ts, f"k{tt}")

            def proj_v(tt, ts):
                psv = pmm.tile([128, 512], f32, tag="mm")
                for c in range(CCH):
                    nc.tensor.matmul(
                        psv[:], lhsT=wv_sb[:, c * 128:(c + 1) * 128],
                        rhs=xchunk(tt, c),
                        start=(c == 0), stop=(c == CCH - 1))
                nc.vector.tensor_copy(v2t[:, ts], psv[:])
                # V transpose to token-major via PE transposes (DMA
                # transposes would serialize against collective SDMA)
                for g in range(2):
                    pst = pmm.tile([128, 256], bf16, tag="mm",
                                   name=f"vt{tt}_{g}")
                    for q in range(4):
                        nc.tensor.transpose(
                            pst[:, q * 64:(q + 1) * 64],
                            v2t[g * 64:(g + 1) * 64,
                                tt * 512 + q * 128:tt * 512 + (q + 1) * 128],
                            ident[g * 64:(g + 1) * 64, g * 64:(g + 1) * 64])
                    v2_dst = bass.AP(v2.tensor,
                                     v2.offset + (4 * tt) * 256 + g * 128,
                                     [v2.ap[0], [256, 4], [1, 64]])
                    nc.vector.tensor_copy(
                        v2_dst, pst[:].rearrange("p (q d) -> p q d", d=64))

            def proj_q(tt, ts, qraw, m):
                psq = pmm.tile([128, 512], f32, tag="mm")
                for c in range(CCH):
                    nc.tensor.matmul(
                        psq[:], lhsT=wqchunk(c, m),
                        rhs=xchunk(tt, c),
                        start=(c == 0), stop=(c == CCH - 1))
                qm = qraw[:, m * 512:(m + 1) * 512]
                nc.vector.tensor_copy(qm, psq[:])
                rope_apply(qq[:, m * L + tt * 512:m * L + (tt + 1) * 512],
                           qm, ts, f"q{tt}_{m}")

            def proj(tt):
                ts = slice(tt * 512, (tt + 1) * 512)
                if tt + 1 < TT:  # prefetch next tile
                    load_x(tt + 1)
                qraw = rope.tile([128, 4 * 512], bf16, tag="qraw")
                if tt == 0:
                    # all of attn(qT=0) is diagonal: K/V first
                    proj_k(tt, ts)
                    proj_v(tt, ts)
                    for m in range(4):
                        proj_q(tt, ts, qraw, m)
                else:
                    # attn(jj=0, qT=tt)'s early key-tiles need only
                    # qq(m=0) + old kk, so the exp stream restarts after
                    # just Qm0+K; V/VT trail (PV of the diagonal kts
                    # lags, covered by deep p bufs)
                    proj_q(tt, ts, qraw, 0)
                    proj_k(tt, ts)
                    for m in range(1, 4):
                        proj_q(tt, ts, qraw, m)
                    proj_v(tt, ts)

            def attn_group(jj, qT):
                """Attention for head pair (jj, jj+4), query tile qT."""
                nkt = 4 * qT + 4
                qoff = jj * L
                qs = slice(qoff + qT * 512, qoff + (qT + 1) * 512)
                o0 = po.tile([128, 512], f32, tag="o", name=f"o0_{jj}_{qT}")
                o1 = po.tile([128, 512], f32, tag="o", name=f"o1_{jj}_{qT}")
                for kt in range(nkt):
                    ksl = slice(kt * 128, (kt + 1) * 128)
                    dj = kt - 4 * qT  # >=0 -> diagonal block
                    cut = 128 * dj if dj >= 0 else 0
                    sb = ps_s.tile([128, 1024], f32, tag="s",
                                   name=f"sb_{jj}_{qT}_{kt}")
                    with tc.high_priority(offset=1000000):
                        nc.tensor.matmul(
                            sb[:, cut:512],
                            lhsT=kk[0:64, ksl],
                            rhs=qq[0:64, qs.start + cut:qs.stop], start=True,
                            stop=(dj < 0), tile_position=(0, 0))
                        nc.tensor.matmul(
                            sb[:, 512 + cut:1024],
                            lhsT=kk[64:128, ksl],
                            rhs=qq[64:128, qs.start + cut:qs.stop],
                            start=True,
                            stop=(dj < 0), tile_position=(64, 0))
                        if dj >= 0:
                            # one MM adds the -48 triangle to BOTH head
                            # halves: rhs repeats bmask via a stride-0
                            # middle dim, dst strides across the banks
                            mdst = bass.AP(sb.tensor, sb.offset + cut,
                                           [sb.ap[0], [512, 2], [1, 128]])
                            mrhs = bass.AP(bmask.tensor, bmask.offset,
                                           [bmask.ap[0], [0, 2], [1, 128]])
                            nc.tensor.matmul(
                                mdst, lhsT=ident[:], rhs=mrhs,
                                start=False, stop=True,
                                skip_group_check=True)
                    p = pp.tile([128, 1024], bf16, tag="p", bufs=10,
                                name=f"p_{jj}_{qT}_{kt}")
                    sb_v = bass.AP(sb.tensor, sb.offset + cut,
                                   [sb.ap[0], [512, 2], [1, 512 - cut]])
                    p_v = bass.AP(p.tensor, p.offset + cut,
                                  [p.ap[0], [512, 2], [1, 512 - cut]])
                    nc.scalar.activation(
                        p_v, sb_v, mybir.ActivationFunctionType.Exp)
                    pvcut = cut if dj > 0 else 0
                    nc.tensor.matmul(
                        o0[:, pvcut:512],
                        lhsT=v2[:, kt * 256:kt * 256 + 128],
                        rhs=p[:, pvcut:512],
                        start=(kt == 0), stop=(kt == nkt - 1))
                    nc.tensor.matmul(
                        o1[:, pvcut:512],
                        lhsT=v2[:, kt * 256 + 128:(kt + 1) * 256],
                        rhs=p[:, 512 + pvcut:1024],
                        start=(kt == 0), stop=(kt == nkt - 1))
                # normalize (approx-recip full tile; rows 64:128 hold the
                # replicated sums - base!=0 slices break the custom-DVE op)
                rc = pp.tile([128, 512], f32, tag="rc", bufs=2,
                             name=f"rc_{jj}_{qT}")
                nc.vector.reciprocal_approx_fast(rc[:], o0[:])
                nc.vector.tensor_tensor(
                    ao[0:64, qs], o0[0:64, :], rc[64:128, :],
                    mybir.AluOpType.mult)
                rc2 = pp.tile([128, 512], f32, tag="rc", bufs=2,
                              name=f"rc2_{jj}_{qT}")
                nc.vector.reciprocal_approx_fast(rc2[:], o1[:])
                nc.vector.tensor_tensor(
                    ao[64:128, qs], o1[0:64, :], rc2[64:128, :],
                    mybir.AluOpType.mult)

            # AG pieces.  Gathered-chunk layout: wo contraction chunk
            # c = 4*jj + rank; within a chunk, row p = 64*g + d holds
            # q-head (8*rank + jj + 4*g), dim d.  All quarters ship whole
            # (bounce row = 128*jj + 64*g + d -> gathered block
            # b = 4*rank + jj): few big AGs keep the serial CC core -- in
            # both the scheduler's cost model and reality -- short.
            bounces = {}
            gaths = {}
            for tq in range(2):
                bounces[tq] = dram.tile([512, 512], bf16, name=f"bounce{tq}")
                gaths[tq] = dram.tile([TP * 512, 512], bf16,
                                      name=f"gath{tq}")
            for tq in (2, 3):
                for jj in range(4):
                    bounces[(tq, jj)] = dram.tile([128, 512], bf16,
                                                  name=f"bounce{tq}{jj}")
                    gaths[(tq, jj)] = dram.tile([TP * 128, 512], bf16,
                                                name=f"gath{tq}{jj}")

            def shipjj(tq, jj):
                """Head pair (jj, jj+4), token tile tq (tq in {2,3})."""
                bnc = bounces[(tq, jj)]
                for g in range(2):
                    nc.sync.dma_start(
                        bnc[64 * g:64 * g + 64, :],
                        ao[g * 64:(g + 1) * 64,
                           jj * L + tq * 512:jj * L + (tq + 1) * 512])
                nc.gpsimd.collective_compute(
                    "AllGather", mybir.AluOpType.bypass,
                    replica_groups=[[0, 1, 2, 3], [4, 5, 6, 7]],
                    ins=[bnc.opt()], outs=[gaths[(tq, jj)].opt()])

            def ship(tq):
                """Bounce ao (all heads, token tile tq) + AllGather."""
                bnc = bounces[tq]
                for g in range(2):
                    for jj in range(4):
                        r0 = 128 * jj + 64 * g
                        nc.sync.dma_start(
                            bnc[r0:r0 + 64, :],
                            ao[g * 64:(g + 1) * 64,
                               jj * L + tq * 512:jj * L + (tq + 1) * 512])
                nc.gpsimd.collective_compute(
                    "AllGather", mybir.AluOpType.bypass,
                    replica_groups=[[0, 1, 2, 3], [4, 5, 6, 7]],
                    ins=[bnc.opt()], outs=[gaths[tq].opt()])

            aoks = {}

            def aok_load(key, nblk, tag, bufs):
                """Load a gathered piece to SBUF (one DMA, gpsimd queue)."""
                aok = aogp.tile([128, nblk * 512], bf16, tag=tag, bufs=bufs,
                                name=f"aok{key}")
                nc.gpsimd.dma_start(
                    aok[:].rearrange("p (b t) -> p b t", t=512),
                    gaths[key][:].rearrange("(b p) t -> p b t", p=128))
                aoks[key] = aok

            def oproj_mms(tq):
                """o-proj for token-quarter tq (chunk c = 4*jj + rank)."""
                ts = slice(tq * 512, (tq + 1) * 512)
                for ct in range(4):
                    pso = pmm.tile([128, 512], f32, tag="mm",
                                   name=f"pso{tq}_{ct}")
                    for jj in range(4):
                        for r in range(TP):
                            c = 4 * jj + r
                            if tq < 2:
                                rhs_t = aoks[tq]
                                b = 4 * r + jj
                            else:
                                rhs_t = aoks[(tq, jj)]
                                b = r
                            nc.tensor.matmul(
                                pso[:],
                                lhsT=wo_sb[:, c * 512 + ct * 128:
                                           c * 512 + (ct + 1) * 128],
                                rhs=rhs_t[:, b * 512:(b + 1) * 512],
                                start=(c == 0), stop=(c == CCH - 1))
                    ost = ostp.tile([128, 512], f32, tag="ost",
                                    name=f"ost{tq}_{ct}")
                    nc.vector.tensor_copy(ost[:], pso[:])
                    nc.gpsimd.dma_start(
                        out[ct * 128:(ct + 1) * 128, ts], ost[:])

            # ================= schedule =================
            for tt in range(TT):
                proj(tt)
                if 1 <= tt <= 2:
                    ship(tt - 1)
                for jj in range(4):
                    attn_group(jj, tt)
                    if tt >= 2:
                        shipjj(tt, jj)
            loopbuf.release()
            aogp = tc.alloc_tile_pool(name="aog", bufs=2)
            aok_load(0, CCH, "aokq", 2)
            oproj_mms(0)
            aok_load(1, CCH, "aokq", 2)
            oproj_mms(1)
            for jj in range(4):
                aok_load((2, jj), TP, "aokp", 8)
            oproj_mms(2)
            for jj in range(4):
                aok_load((3, jj), TP, "aokp", 8)
            oproj_mms(3)
            aogp.release()

    nc.compile()
    return nc


def _host_prep(hidden_states, cos, sin, Wq, Wk, Wv, Wo):
    """Build the 8 per-core input maps (all host-side packing)."""
    scale = float(D) ** -0.5
    # rope coefficient tables [128, L]: 4 groups of 32 rows (d 0:32 pattern)
    cosT = cos[:, :32].T.astype(np.float32)          # [32, L]
    sinT = sin[:, :32].T.astype(np.float32)
    c1 = np.tile(cosT, (4, 1))                       # [128, L]
    c2 = np.concatenate([-sinT, sinT, -sinT, sinT], axis=0)
    # rope half-swap permutation (d -> d XOR 32), as a PE stationary matrix
    pm = np.zeros((128, 128), np.float32)
    pm[np.arange(128) ^ 32, np.arange(128)] = 1.0
    tables = {"C1": c1.astype(BF16), "C2": c2.astype(BF16),
              "PM": pm.astype(BF16)}

    def pack(WT, m):
        # WT [HID, m] -> [128, CCH*m] with row p = concat_c WT[c*128+p, :]
        return np.ascontiguousarray(
            WT.reshape(CCH, 128, m).transpose(1, 0, 2).reshape(128, CCH * m)
        ).astype(BF16)

    # x packed per (tt, c): [128, tt, c, 512]
    xPb = []
    for b in range(B):
        xT = hidden_states[b].T.astype(np.float32)   # [HID, L]
        xp = (xT.reshape(CCH, 128, TT, 512).transpose(1, 2, 0, 3)
              .reshape(128, TT * CCH * 512))
        xPb.append(np.ascontiguousarray(xp).astype(BF16))

    # o-proj contraction-row order: R -> chunk c = R//128 = 4*jj + rank,
    # row p = R%128 = 64*g + d, holding q-head (8*rank + jj + 4*g)
    RR = np.arange(2048)
    perm = ((8 * ((RR // 128) % 4) + RR // 512 + 4 * ((RR % 128) // 64)) * D
            + RR % 64)

    in_maps = []
    for i in range(N_CORES):
        b, r = divmod(i, TP)
        # Wq rows reordered: M-tile m = heads (8r+m, 8r+4+m); scale folded in
        rows = []
        for m in range(4):
            rows.append(Wq[(8 * r + m) * D:(8 * r + m + 1) * D])
            rows.append(Wq[(8 * r + 4 + m) * D:(8 * r + 4 + m + 1) * D])
        WqT_i = (np.concatenate(rows, 0) * scale).T.astype(np.float32)
        WkT_i = Wk[2 * r * D:(2 * r + 2) * D].T.astype(np.float32)
        WvT_i = Wv[2 * r * D:(2 * r + 2) * D].T.astype(np.float32)
        WoT_i = Wo[CW * r:CW * (r + 1), :].T[perm].astype(np.float32)
        in_maps.append({
            "xP": xPb[b],
            "WqP": pack(WqT_i, 512),
            "WkP": pack(WkT_i, 128),
            "WvP": pack(WvT_i, 128),
            "WoP": pack(WoT_i, 512),
            **tables,
        })
    return in_maps


def kernel(hidden_states, cos, sin, Wq, Wk, Wv, Wo, _want_profile=False):
    from concourse.bass_utils import run_bass_kernel_spmd

    if "nc" not in _cache:
        _cache["nc"] = _build_graph()
    nc = _cache["nc"]
    in_maps = _host_prep(np.asarray(hidden_states), np.asarray(cos),
                         np.asarray(sin), np.asarray(Wq), np.asarray(Wk),
                         np.asarray(Wv), np.asarray(Wo))
    res = run_bass_kernel_spmd(nc, in_maps, list(range(N_CORES)),
                               trace=_want_profile)
    # assemble: core (b, r) holds out^T [512, L] = cols [512r, 512r+512) of b
    full = np.empty((B, L, HID), np.float32)
    for i in range(N_CORES):
        b, r = divmod(i, TP)
        full[b, :, CW * r:CW * (r + 1)] = res.results[i]["out"].T
    if _want_profile:
        return full, res
    return full
